# revision 83
# baseline (speedup 1.0000x reference)
"""Multi-head attention (B=2, N=2048, d_model=1024, H=16) on 8 NeuronCores.

Sharding: data-parallel on batch (2) x tensor-parallel on heads (4 groups of
4 heads). Core c handles batch c//4, head-group c%4. Each core computes its
heads' Q/K/V projections, causal attention, and a partial output projection;
the host sums the 4 partials per batch and adds bo.

Mixed precision, chosen so the ACT engine's softmax exp (~58us of column
time, dtype-independent) is the bottleneck and every matmul hides under it:
  - Q/K/V projections and the output projection run in fp8e4m3 DoubleRow
    mode (2 contraction k-tiles per instruction at 0.5 cycles/row).
  - Rows that softmax cannot average (queries/keys < 256) stay bf16:
    bf16 projections for those columns, a bf16 PV path, bf16 outproj rows.
  - Scores (S = q.k^T) are computed in bf16; exp writes p directly as fp8
    (bf16 for the early-query block).
  - PV contracts keys with fp8 DoubleRow pairs of key-blocks; the stationary
    v tile carries a ones column so the softmax denominator accumulates in
    PSUM row 64 for free (rows 65..127 are junk and never read).
  - The causal mask is applied by an accumulating PE matmul (stationary
    -1e9*I, moving strict-lower-triangular ones) into the score PSUM before
    exp, so no vector-engine mask multiplies are needed.

Measured numerics for this exact scheme: relmax ~5.9e-3 against the fp32
reference (gate 2e-2). TimelineSim exec: ~98.4us (baseline 141.5us).
"""

import sys

if "/opt/trn_rl_repo" not in sys.path:
    sys.path.insert(0, "/opt/trn_rl_repo")

import numpy as np
import ml_dtypes

import concourse.bass as bass
import concourse.mybir as mybir
import concourse.tile as tile
from concourse import bacc
from concourse.bass_utils import run_bass_kernel_spmd
from concourse.masks import make_lower_triangular, make_identity

B, N, D, H = 2, 2048, 1024, 16
DV = D // H  # 64
HPC = H // 4  # heads per core: 4
DHC = HPC * DV  # head dims per core: 256
DT = D // 128  # 8 din-tiles
NE = 256  # early boundary: queries/keys < NE use bf16 paths
BF = mybir.dt.bfloat16
F8 = mybir.dt.float8e4
F16 = mybir.dt.float16
F32 = mybir.dt.float32
EXP = mybir.ActivationFunctionType.Exp
DR = mybir.MatmulPerfMode.DoubleRow
SCALE = 0.125  # 1/sqrt(DV)
NEG = -1.0e9

_CACHE = {}


def build_nc():
    nc = bacc.Bacc("TRN2", target_bir_lowering=False, debug=False)
    xq8_d = nc.dram_tensor("xq8", [D, N], F8, kind="ExternalInput")
    xk8_d = nc.dram_tensor("xk8", [D, N], F8, kind="ExternalInput")
    xv8_d = nc.dram_tensor("xv8", [D, N], F8, kind="ExternalInput")
    xqb_d = nc.dram_tensor("xqb", [D, NE], BF, kind="ExternalInput")
    xkb_d = nc.dram_tensor("xkb", [D, NE], BF, kind="ExternalInput")
    xvb_d = nc.dram_tensor("xvb", [D, NE], BF, kind="ExternalInput")
    wq8_d = nc.dram_tensor("wq8", [D, DHC], F8, kind="ExternalInput")
    wqb_d = nc.dram_tensor("wqb", [D, DHC], BF, kind="ExternalInput")
    wo8_d = nc.dram_tensor("wo8", [DHC, D], F8, kind="ExternalInput")
    wob_d = nc.dram_tensor("wob", [DHC, D], BF, kind="ExternalInput")
    bq_d = nc.dram_tensor("bq", [DHC], F32, kind="ExternalInput")
    yT_d = nc.dram_tensor("yT", [D, N], F16, kind="ExternalOutput")

    with tile.TileContext(nc) as tc:
        with (
            tc.tile_pool(name="consts", bufs=1) as consts,
            tc.tile_pool(name="xin", bufs=1) as xin,
            tc.tile_pool(name="prod", bufs=1) as prod,
            tc.tile_pool(name="pp8", bufs=1) as pp8,
            tc.tile_pool(name="norm", bufs=3) as norm,
            tc.tile_pool(name="yout", bufs=2) as yout,
            tc.tile_pool(name="ps", bufs=1, space="PSUM") as ps,
        ):
            # ---- weights + constants ----
            wq8t = consts.tile([128, DT, DHC], F8, name="wq8t")
            nc.sync.dma_start(
                out=wq8t, in_=wq8_d.ap().rearrange("(j p) c -> p j c", p=128)
            )
            wqbt = consts.tile([128, DT, DHC], BF, name="wqbt")
            bq_pp = consts.tile([128, 2], F32, name="bq_pp")
            bq_row = consts.tile([1, DHC], F32, name="bq_row")
            bq_bc = consts.tile([128, DHC], F32, name="bq_bc")
            # strict lower-triangular -1e9 (row k, col q: NEG iff k > q),
            # doubled along a middle dim so one mask matmul covers 2 heads
            ltm = consts.tile([128, 2, 128], BF, name="ltm")
            make_lower_triangular(nc, ltm[:, 0, :], val=NEG, diag=False)
            nc.vector.tensor_copy(ltm[:, 1, :], ltm[:, 0, :])
            negeye = consts.tile([128, 128], BF, name="negeye")
            make_identity(nc, negeye)

            # ---- bulk inputs ----
            xq8t = xin.tile([128, DT, N], F8, name="xq8t")
            xk8t = xin.tile([128, DT, N], F8, name="xk8t")
            xv8t = xin.tile([128, DT, N], F8, name="xv8t")
            xqbt = xin.tile([128, DT, NE], BF, name="xqbt")
            xkbt = xin.tile([128, DT, NE], BF, name="xkbt")
            xvbt = xin.tile([128, DT, NE], BF, name="xvbt")

            def load_slice(t, d, n0, n1):
                nc.sync.dma_start(
                    out=t[:, :, n0:n1],
                    in_=d.ap()[:, n0:n1].rearrange("(j p) n -> p j n", p=128),
                )

            # The sim's DMA device is serial (~350B/ns), so transfers are
            # strictly need-ordered. The first exp is gated on the chunk-0
            # q projection, so q loads lead.
            nc.sync.dma_start(
                out=bq_pp, in_=bq_d.ap().rearrange("(c p) -> p c", p=128)
            )
            load_slice(xk8t, xk8_d, 0, 512)
            load_slice(xq8t, xq8_d, 0, 512)
            load_slice(xq8t, xq8_d, 1536, N)
            load_slice(xk8t, xk8_d, 512, N)
            load_slice(xkbt, xkb_d, 0, NE)
            load_slice(xqbt, xqb_d, 0, NE)
            nc.sync.dma_start(
                out=bq_row, in_=bq_d.ap().rearrange("(a c) -> a c", a=1)
            )
            nc.gpsimd.partition_broadcast(bq_bc, bq_row)
            nc.sync.dma_start(
                out=wqbt, in_=wqb_d.ap().rearrange("(j p) c -> p j c", p=128)
            )
            wo8t = consts.tile([128, 2, D], F8, name="wo8t")
            nc.sync.dma_start(
                out=wo8t, in_=wo8_d.ap().rearrange("(s p) c -> p s c", p=128)
            )
            wobt = consts.tile([128, 2, D], BF, name="wobt")
            nc.sync.dma_start(
                out=wobt, in_=wob_d.ap().rearrange("(s p) c -> p s c", p=128)
            )
            load_slice(xv8t, xv8_d, 0, 512)
            load_slice(xvbt, xvb_d, 0, NE)
            load_slice(xq8t, xq8_d, 1024, 1536)
            load_slice(xv8t, xv8_d, 512, N)
            load_slice(xq8t, xq8_d, 512, 1024)

            # ---- persistent products ----
            # qT/kT: bf16, [128, N] per head-pair hp (head 2hp on partitions
            # 0:64, head 2hp+1 on 64:128)
            qT = [prod.tile([128, N], BF, name=f"qT{p}") for p in range(2)]
            kT = [prod.tile([128, N], BF, name=f"kT{p}") for p in range(2)]
            # v, PV-stationary layout: [keys(128), key-block m, head, 128]
            # cols 0:64 = v dims, col 64 = ones (denominator), 65:128 junk
            v8p = prod.tile([128, 16, HPC, 128], F8, name="v8p")
            vbp = prod.tile([128, 2, HPC, 128], BF, name="vbp")
            # cols 64:128 all-ones => PV psum rows 64:128 hold the softmax
            # denominator replicated across 64 partitions (broadcast-free
            # normalization)
            nc.gpsimd.memset(v8p[:, :, :, 64:128], 1.0)
            nc.gpsimd.memset(vbp[:, :, :, 64:128], 1.0)
            # attention outputs (x^T): dm = 64h+dv -> partition (64h+dv)%128,
            # subtile (64h+dv)//128
            xa8 = prod.tile([128, 2, N], F8, name="xa8")
            xab = prod.tile([128, 2, NE], BF, name="xab")
            # p tiles: per chunk [keys, j, head, 512 chunk-local queries]
            pbf = prod.tile([128, 2, HPC, NE], BF, name="pbf")

            def p8_tile():
                return pp8.tile(
                    [128, 16, HPC, 512], F8, name="p8", tag="p8", bufs=2
                )

            # ---- building blocks ----
            def proj_qk(c, p, which, lo=0, hi=512, tag="prj", act=False):
                """fp8 DoubleRow q/k projection for chunk c, half p,
                column range [lo, hi) within the chunk. act=True drains the
                psum on the ACT engine (idle during the prologue)."""
                src8 = xq8t if which == "q" else xk8t
                dst = qT if which == "q" else kT
                pp = ps.tile([128, 512], F32, name="pp", tag=tag, bufs=2)
                n0 = c * 512
                for t in range(DT // 2):
                    nc.tensor.matmul(
                        pp[:, lo:hi],
                        wq8t[:, 2 * t : 2 * t + 2, p * 128 : (p + 1) * 128],
                        src8[:, 2 * t : 2 * t + 2, n0 + lo : n0 + hi],
                        start=(t == 0),
                        stop=(t == DT // 2 - 1),
                        perf_mode=DR,
                        skip_group_check=True,
                    )
                if act:
                    nc.scalar.activation(
                        dst[p][:, n0 + lo : n0 + hi],
                        pp[:, lo:hi],
                        mybir.ActivationFunctionType.Identity,
                        bias=bq_pp[:, p : p + 1],
                    )
                else:
                    nc.vector.tensor_scalar_add(
                        dst[p][:, n0 + lo : n0 + hi],
                        pp[:, lo:hi],
                        bq_pp[:, p : p + 1],
                    )

            def proj_qkb_fix(p, which):
                """bf16 redo of the first NE query/key columns (overwrites
                the fp8-projected values for softmax-sensitive early rows)."""
                srcb = xqbt if which == "q" else xkbt
                dst = qT if which == "q" else kT
                pp = ps.tile([128, 512], F32, name="pp", tag="prj", bufs=2)
                for j in range(DT):
                    nc.tensor.matmul(
                        pp[:, 0:NE],
                        wqbt[:, j, p * 128 : (p + 1) * 128],
                        srcb[:, j, :],
                        start=(j == 0),
                        stop=(j == DT - 1),
                        skip_group_check=True,
                    )
                nc.vector.tensor_scalar_add(
                    dst[p][:, 0:NE], pp[:, 0:NE], bq_pp[:, p : p + 1]
                )

            def proj_v(m):
                """Project v for key-block m into v8p (fp8) and, for m<2,
                vbp (bf16 from bf16 operands)."""
                pv = ps.tile([128, 512], F32, name="pv", tag="prj", bufs=2)
                pvv = pv[:, 0:DHC]
                for t in range(DT // 2):
                    nc.tensor.matmul(
                        pvv,
                        xv8t[:, 2 * t : 2 * t + 2, m * 128 : (m + 1) * 128],
                        wq8t[:, 2 * t : 2 * t + 2, :],
                        start=(t == 0),
                        stop=(t == DT // 2 - 1),
                        perf_mode=DR,
                        skip_group_check=True,
                    )
                nc.vector.tensor_add(
                    v8p[:, m, :, 0:DV],
                    pvv.rearrange("p (h d) -> p h d", h=HPC),
                    bq_bc.rearrange("p (h d) -> p h d", h=HPC),
                )
                if m < 2:
                    pb = ps.tile([128, 512], F32, name="pb", tag="prj", bufs=2)
                    pbb = pb[:, 0:DHC]
                    for j in range(DT):
                        nc.tensor.matmul(
                            pbb,
                            xvbt[:, j, m * 128 : (m + 1) * 128],
                            wqbt[:, j, :],
                            start=(j == 0),
                            stop=(j == DT - 1),
                            skip_group_check=True,
                        )
                    nc.vector.tensor_add(
                        vbp[:, m, :, 0:DV],
                        pbb.rearrange("p (h d) -> p h d", h=HPC),
                        bq_bc.rearrange("p (h d) -> p h d", h=HPC),
                    )

            def s_group(c, j, hp, p8c, fillers, split=False):
                """Scores+exp (fp8 region) for (chunk c, key block j, head
                pair hp). For c==0 the region starts at NE; queries < NE are
                handled by s_fix with bf16-projected q/k."""
                off = max(0, (j - 4 * c) * 128)
                lo = max(off, NE) if c == 0 else off
                n0 = c * 512
                diag = j >= 4 * c and off >= lo
                sp = ps.tile([128, 1024], F32, name="sp", tag="sp", bufs=2)
                for hr in range(2):
                    nc.tensor.matmul(
                        sp[:, hr * 512 + lo : (hr + 1) * 512],
                        kT[hp][
                            hr * 64 : (hr + 1) * 64, j * 128 : (j + 1) * 128
                        ],
                        qT[hp][hr * 64 : (hr + 1) * 64, n0 + lo : n0 + 512],
                        start=True,
                        stop=not diag,
                        skip_group_check=True,
                    )
                spv = sp.rearrange("p (b k) -> p b k", b=2)
                if diag:
                    # diagonal block: add -1e9 where key > query
                    nc.tensor.matmul(
                        spv[:, :, off : off + 128],
                        negeye,
                        ltm,
                        start=False,
                        stop=True,
                        skip_group_check=True,
                    )
                for f in fillers:
                    f()
                if split:
                    # per-head exps so the first head's PV/norm can start
                    # while the second head's exp still runs
                    for hr in range(2):
                        nc.scalar.activation(
                            p8c[:, j, 2 * hp + hr, lo:512],
                            spv[:, hr, lo:512],
                            EXP,
                            scale=SCALE,
                        )
                else:
                    nc.scalar.activation(
                        p8c[:, j, 2 * hp : 2 * hp + 2, lo:512],
                        spv[:, :, lo:512],
                        EXP,
                        scale=SCALE,
                    )

            def s_fix(j, hp):
                """Early-query scores (queries < NE) from the bf16-projected
                q/k, exp'd into the bf16 p tile for the bf16 PV path."""
                off = j * 128
                sp = ps.tile([128, 1024], F32, name="sp", tag="sp", bufs=2)
                for hr in range(2):
                    nc.tensor.matmul(
                        sp[:, hr * 512 + off : hr * 512 + NE],
                        kT[hp][
                            hr * 64 : (hr + 1) * 64, j * 128 : (j + 1) * 128
                        ],
                        qT[hp][hr * 64 : (hr + 1) * 64, off:NE],
                        start=True,
                        stop=False,
                        skip_group_check=True,
                    )
                spv = sp.rearrange("p (b k) -> p b k", b=2)
                nc.tensor.matmul(
                    spv[:, :, off : off + 128],
                    negeye,
                    ltm,
                    start=False,
                    stop=True,
                    skip_group_check=True,
                )
                nc.scalar.activation(
                    pbf[:, j, 2 * hp : 2 * hp + 2, off:NE],
                    spv[:, :, off:NE],
                    EXP,
                    scale=SCALE,
                )

            def pv_head(c, h, p8c, op, qlo=0, qhi=512):
                """PV + denominator for (chunk c, head h), query columns
                [qlo, qhi) of the chunk -> op psum cols [qlo, qhi).
                op rows 0:64 = x dims, rows 64:128 = denominator."""
                j0 = 4 * c
                if c == 0:
                    nc.tensor.matmul(
                        op[:, 0:NE],
                        vbp[:, 0, h, :],
                        pbf[:, 0, h, 0:NE],
                        start=True,
                        stop=False,
                        skip_group_check=True,
                    )
                    nc.tensor.matmul(
                        op[:, 128:NE],
                        vbp[:, 1, h, :],
                        pbf[:, 1, h, 128:NE],
                        start=False,
                        stop=True,
                        skip_group_check=True,
                    )
                    nc.tensor.matmul(
                        op[:, NE:512],
                        v8p[:, 0:2, h, :],
                        p8c[:, 0:2, h, NE:512],
                        start=True,
                        stop=False,
                        perf_mode=DR,
                        skip_group_check=True,
                    )
                    nc.tensor.matmul(
                        op[:, NE:384],
                        v8p[:, 2, h, :],
                        p8c[:, 2, h, NE:384],
                        start=False,
                        stop=False,
                        skip_group_check=True,
                    )
                    nc.tensor.matmul(
                        op[:, 384:512],
                        v8p[:, 2:4, h, :],
                        p8c[:, 2:4, h, 384:512],
                        start=False,
                        stop=True,
                        perf_mode=DR,
                        skip_group_check=True,
                    )
                    return
                # instruction list for the causal trapezoid, each clipped
                # to [qlo, qhi): (j_start, n_j, col_lo, col_hi)
                instrs = [(2 * p, 2, 0, 512) for p in range(2 * c)]
                instrs += [
                    (j0, 1, 0, 128),
                    (j0, 2, 128, 512),
                    (j0 + 2, 1, 256, 384),
                    (j0 + 2, 2, 384, 512),
                ]
                clipped = []
                for js, nj, lo, hi in instrs:
                    lo, hi = max(lo, qlo), min(hi, qhi)
                    if lo < hi:
                        clipped.append((js, nj, lo, hi))
                for i, (js, nj, lo, hi) in enumerate(clipped):
                    nc.tensor.matmul(
                        op[:, lo:hi],
                        v8p[:, js : js + nj, h, :],
                        p8c[:, js : js + nj, h, lo:hi],
                        start=(i == 0),
                        stop=(i == len(clipped) - 1),
                        perf_mode=DR if nj == 2 else None,
                        skip_group_check=True,
                    )

            def norm_head(c, h, op, qlo=0, qhi=512):
                """Normalize op rows 0:64 by the denominator (replicated in
                rows 64:128), write x^T tiles."""
                n0 = c * 512
                w = qhi - qlo
                rrec = norm.tile([64, 512], F32, name="rrec", tag="rrec")
                nc.vector.reciprocal(rrec[:, 0:w], op[64:128, qlo:qhi])
                pbase = (h % 2) * 64
                s = h // 2
                if c == 0:
                    nc.vector.tensor_mul(
                        xab[pbase : pbase + 64, s, :],
                        op[0:DV, 0:NE],
                        rrec[:, 0:NE],
                    )
                    nc.vector.tensor_mul(
                        xa8[pbase : pbase + 64, s, NE:512],
                        op[0:DV, NE:512],
                        rrec[:, NE:512],
                    )
                else:
                    nc.vector.tensor_mul(
                        xa8[pbase : pbase + 64, s, n0 + qlo : n0 + qhi],
                        op[0:DV, qlo:qhi],
                        rrec[:, 0:w],
                    )

            def outproj(c, t, act_copy=False, tail=False, qlo=0, qhi=512, sub=""):
                """Output projection for chunk c, dout tile t, query columns
                [qlo, qhi)."""
                n0 = c * 512
                w = qhi - qlo
                if tail and t % 3 == 1:
                    # S and PV psum rings are idle in the tail — borrow them
                    # so six outproj psums pipeline instead of two
                    yp = ps.tile([128, 1024], F32, name="sp", tag="sp", bufs=2)
                    yp = yp[:, 0:w]
                elif tail and t % 3 == 2:
                    yp = ps.tile([128, 512], F32, name="op", tag="op", bufs=2)
                    yp = yp[:, 0:w]
                else:
                    yp = ps.tile([128, 512], F32, name="yp", tag="prj", bufs=2)
                    yp = yp[:, 0:w]
                if c == 0:
                    for s in range(2):
                        nc.tensor.matmul(
                            yp[:, 0:NE],
                            wobt[:, s, t * 128 : (t + 1) * 128],
                            xab[:, s, :],
                            start=(s == 0),
                            stop=(s == 1),
                            skip_group_check=True,
                        )
                    nc.tensor.matmul(
                        yp[:, NE:512],
                        wo8t[:, :, t * 128 : (t + 1) * 128],
                        xa8[:, :, NE:512],
                        start=True,
                        stop=True,
                        perf_mode=DR,
                        skip_group_check=True,
                    )
                else:
                    nc.tensor.matmul(
                        yp,
                        wo8t[:, :, t * 128 : (t + 1) * 128],
                        xa8[:, :, n0 + qlo : n0 + qhi],
                        start=True,
                        stop=True,
                        perf_mode=DR,
                        skip_group_check=True,
                    )
                if tail:
                    # pair-tile staging: two dout tiles share one DMA so the
                    # tail's serial HWDGE issue chain halves
                    pair = yout.tile(
                        [128, 2, w], F16, name=f"y_pr{t // 2}",
                        tag=f"yzp{t // 2}", bufs=1,
                    )
                    y_sb = pair[:, t % 2, :]
                else:
                    y_sb = yout.tile(
                        [128, w], F16, name=f"y_sb{sub}{t}",
                        tag=f"yz{sub}{t}" if sub else f"y{t % 4}",
                        bufs=1 if sub else 2,
                    )
                if act_copy:
                    nc.scalar.copy(y_sb, yp)
                else:
                    nc.vector.tensor_copy(y_sb, yp)
                if tail:
                    if t % 2:
                        # defer the DMA: issued after all copies so scalar-
                        # queue dispatches don't block remaining ACT copies
                        t0 = t - 1
                        dst = yT_d.ap()[
                            t0 * 128 : (t0 + 2) * 128, n0 + qlo : n0 + qhi
                        ].rearrange("(s p) n -> p s n", p=128)
                        deferred_dmas.append((dst, pair, t // 2))
                else:
                    # keep mid-kernel output DMAs off the ACT sequencer
                    # (a scalar-queue dispatch costs it 667ns mid-stream)
                    dst = yT_d.ap()[
                        t * 128 : (t + 1) * 128, n0 + qlo : n0 + qhi
                    ]
                    nc.sync.dma_start(out=dst, in_=y_sb)

            def pv_norm(pc, h, pp8c):
                op = ps.tile([128, 512], F32, name="op", tag="op", bufs=2)
                pv_head(pc, h, pp8c, op)
                norm_head(pc, h, op)

            deferred_dmas = []

            def F(fn, *a):
                return lambda: fn(*a)

            # ---- schedule ----
            # Units (chunks) in order 0, 3, 2, 1. Per unit: S+exp groups
            # with PE fillers (projections, then previous unit's PV/norm/
            # outproj) spread between them.
            # q first (it gates the first exp), then k keys 0:128 for the
            # first S group, then the rest. fp8 q cols 0:NE are never read
            # (the bf16 fixup overwrites them), so skip their projection.
            for p in range(2):
                proj_qk(0, p, "k", tag="op", act=True)
                proj_qk(0, p, "q", lo=NE)

            unit_order = [0, 3, 2, 1]

            def unit_fillers(u):
                """PE filler ops to sprinkle through unit u's S groups,
                ordered by DMA arrival of their inputs."""
                fs = []
                if u == 0:  # during chunk 0
                    for p in range(2):
                        fs.append(F(proj_qk, 3, p, "q"))
                elif u == 1:  # during chunk 3
                    for c in (1, 2, 3):
                        for p in range(2):
                            fs.append(F(proj_qk, c, p, "k"))
                    for w in ("k", "q"):
                        for p in range(2):
                            fs.append(F(proj_qkb_fix, p, w))
                    for j in range(2):
                        for hp in range(2):
                            fs.append(F(s_fix, j, hp))
                    for m in range(4):
                        fs.append(F(proj_v, m))
                    for p in range(2):
                        fs.append(F(proj_qk, 2, p, "q"))
                    for m in range(4, 16):
                        fs.append(F(proj_v, m))
                elif u == 2:  # during chunk 2
                    for p in range(2):
                        fs.append(F(proj_qk, 1, p, "q"))
                return fs

            prev = None  # (c, p8c) of previous unit
            for ui, c in enumerate(unit_order):
                p8c = p8_tile()
                fs = unit_fillers(ui)
                # previous unit's consumers are fillers too
                if prev is not None:
                    pc, pp8c = prev
                    for h in range(HPC):
                        fs.append(F(lambda pc=pc, h=h, pp8c=pp8c: pv_norm(
                            pc, h, pp8c)))
                    for t in range(DT):
                        fs.append(F(outproj, pc, t))
                last = ui == len(unit_order) - 1
                if last:
                    # head-pair-major so PV/norm of heads 0,1 can run
                    # during the second half's exps
                    groups = [
                        (j, hp) for hp in range(2) for j in range(4 * c + 4)
                    ]
                else:
                    groups = [
                        (j, hp) for j in range(4 * c + 4) for hp in range(2)
                    ]
                # in the last unit, consume fillers early so DVE's norm/copy
                # work drains before the tail; elsewhere spread evenly
                ng = max(1, (len(groups) * 3) // 4) if last else len(groups)
                nf = len(fs)
                done = 0
                for gi, (j, hp) in enumerate(groups):
                    take = (nf * min(gi + 1, ng)) // ng - done
                    done += take
                    fl, fs = fs[:take], fs[take:]
                    s_group(c, j, hp, p8c, fl,
                            split=(last and hp == 1 and j == 4 * c + 3))
                    if last and hp == 1:
                        # heads 0,1 are fully exp'd after the hp=0 sweep;
                        # overlap their PV under the head-2,3 exps. Heads
                        # 2,3 split by column half: the first half (and its
                        # outproj) runs under the last two groups' exps.
                        if j in (0, 2):
                            pv_norm(c, j // 2 if j else 0, p8c)
                for f in fs:
                    f()
                prev = (c, p8c)

            # tail: last unit's heads 2,3 second column half. ACT is idle
            # now, so alternate psum->sbuf copies DVE/ACT.
            pc, pp8c = prev
            for h in (2, 3):
                pv_norm(pc, h, pp8c)
            for t in range(DT):
                # DVE still drains the norm mults when t=0,1 land — give
                # those copies to the idle ACT engine; balance 4/4
                # each staging pair (2t, 2t+1) splits its two copies
                # across ACT/DVE so the pair's DMA is gated by neither
                # engine's queue alone
                outproj(pc, t, act_copy=(t % 2 == 0), tail=True)
            for dst, y_sb, t in deferred_dmas:
                eng = nc.scalar if t % 2 == 0 else nc.sync
                eng.dma_start(out=dst, in_=y_sb)
    nc.compile()
    return nc


def kernel(**inputs):
    inputs = {k: np.asarray(v) for k, v in inputs.items()}
    Q, K, V = inputs["Q"], inputs["K"], inputs["V"]
    wq, bq, wo, bo = inputs["wq"], inputs["bq"], inputs["wo"], inputs["bo"]
    bf = ml_dtypes.bfloat16
    f8 = ml_dtypes.float8_e4m3fn

    def T8(x):
        return np.ascontiguousarray(x.astype(f8).T)

    def Tb(x):
        return np.ascontiguousarray(x.astype(bf).T)

    xq8 = [T8(Q[b]) for b in range(B)]
    xk8 = [T8(K[b]) for b in range(B)]
    xv8 = [T8(V[b]) for b in range(B)]
    xqb = [Tb(Q[b, :NE]) for b in range(B)]
    xkb = [Tb(K[b, :NE]) for b in range(B)]
    xvb = [Tb(V[b, :NE]) for b in range(B)]
    wq8 = [T8(wq[g * DHC : (g + 1) * DHC, :]) for g in range(4)]
    wqb = [Tb(wq[g * DHC : (g + 1) * DHC, :]) for g in range(4)]
    wo8 = [T8(wo[:, g * DHC : (g + 1) * DHC]) for g in range(4)]
    wob = [Tb(wo[:, g * DHC : (g + 1) * DHC]) for g in range(4)]
    bqs = [
        np.ascontiguousarray(bq[g * DHC : (g + 1) * DHC], dtype=np.float32)
        for g in range(4)
    ]

    if "nc" not in _CACHE:
        _CACHE["nc"] = build_nc()
    nc = _CACHE["nc"]

    in_maps = []
    for core in range(8):
        b, g = divmod(core, 4)
        in_maps.append(
            {
                "xq8": xq8[b],
                "xk8": xk8[b],
                "xv8": xv8[b],
                "xqb": xqb[b],
                "xkb": xkb[b],
                "xvb": xvb[b],
                "wq8": wq8[g],
                "wqb": wqb[g],
                "wo8": wo8[g],
                "wob": wob[g],
                "bq": bqs[g],
            }
        )
    import os

    trace = bool(int(os.environ.get("KERNEL_TRACE", "0")))
    try:
        res = run_bass_kernel_spmd(
            nc, in_maps, core_ids=list(range(8)), trace=trace
        )
    except ModuleNotFoundError:
        res = run_bass_kernel_spmd(nc, in_maps, core_ids=list(range(8)))
    _CACHE["last_results"] = res

    out = np.empty((B, N, D), np.float32)
    for b in range(B):
        acc = res.results[4 * b]["yT"].astype(np.float32)
        for g in range(1, 4):
            acc += res.results[4 * b + g]["yT"]
        out[b] = acc.T + bo
    return out
